# revision 7
# baseline (speedup 1.0000x reference)
"""Fused Conv3x3+BN+LeakyReLU -> QKV -> spatial self-attention -> residual+LN+LeakyReLU
Trainium2 Bass kernel, data-parallel over batch on 8 NeuronCores.

Layout strategy (per core, 4 samples):
  Features kept "c-major": [channel (2 x 128 partitions), pixel (free)].
  Conv as 9-tap matmul accumulation over a spatially padded input.
  BatchNorm statistics AllReduced across cores (batch axis is sharded).
  Attention computed per sample with softmax along the free axis skipped
  max-subtraction (scores are O(5)); Z (softmax denominators) obtained with a
  ones-matmul that also broadcasts across partitions.
"""
import sys
import numpy as np

sys.path.insert(0, "/opt/trn_rl_repo")

N_CORES = 8
S = 4            # samples per core
H = W = 32
C = 256
NPIX = S * H * W            # 4096 pixels per core
HP = H + 2                  # padded
PS = HP * HP                # padded pixels per sample (1156)
ALPHA = 0.3
BN_EPS = 1e-3
LN_EPS = 1e-3

_CACHE = {}


def _build():
    import concourse.bass as bass
    import concourse.bacc as bacc
    import concourse.tile as tile
    import concourse.mybir as mybir
    from concourse.masks import make_identity

    F32 = mybir.dt.float32
    F32R = mybir.dt.float32r
    BF16 = mybir.dt.bfloat16
    AF = mybir.ActivationFunctionType
    OP = mybir.AluOpType

    nc = bacc.Bacc("TRN2", target_bir_lowering=False, debug=False,
                   num_devices=N_CORES)

    x_s = nc.declare_dram_parameter("x_s", [NPIX, C], F32, isOutput=False)
    w_cbl = nc.declare_dram_parameter("w_cbl", [3, 3, C, C], F32, isOutput=False)
    b_cbl = nc.declare_dram_parameter("b_cbl", [C], F32, isOutput=False)
    bn_gamma = nc.declare_dram_parameter("bn_gamma", [C], F32, isOutput=False)
    bn_beta = nc.declare_dram_parameter("bn_beta", [C], F32, isOutput=False)
    wq = nc.declare_dram_parameter("wq", [C, C], F32, isOutput=False)
    bq = nc.declare_dram_parameter("bq", [C], F32, isOutput=False)
    wk = nc.declare_dram_parameter("wk", [C, C], F32, isOutput=False)
    bk = nc.declare_dram_parameter("bk", [C], F32, isOutput=False)
    wv = nc.declare_dram_parameter("wv", [C, C], F32, isOutput=False)
    bv = nc.declare_dram_parameter("bv", [C], F32, isOutput=False)
    ln_gamma = nc.declare_dram_parameter("ln_gamma", [H * W, C], F32, isOutput=False)
    ln_beta = nc.declare_dram_parameter("ln_beta", [H * W, C], F32, isOutput=False)
    y_s = nc.declare_dram_parameter("y_s", [NPIX, C], F32, isOutput=True)

    def ecopy(i, out, in_):
        if i % 2 == 0:
            nc.vector.tensor_copy(out, in_)
        else:
            nc.scalar.copy(out, in_)

    with tile.TileContext(nc) as tc:
        import contextlib
        est = contextlib.ExitStack()
        with est:
            persist = est.enter_context(tc.tile_pool(name="persist", bufs=1))
            pstat = est.enter_context(tc.tile_pool(name="pstat", bufs=1))
            dram = est.enter_context(tc.tile_pool(name="dram", bufs=1, space="DRAM"))

            # ---------- persistent small constants ----------
            ident = persist.tile([128, 128], F32, tag="ident")
            make_identity(nc, ident[:])
            ones_bf = persist.tile([128, 128], BF16, tag="ones_bf")
            nc.vector.memset(ones_bf[:], 1.0)
            ones1f = persist.tile([1, 128], F32, tag="ones1f")
            nc.vector.memset(ones1f[:], 1.0)
            ones1r = persist.tile([1, 128], F32R, tag="ones1r")
            nc.vector.tensor_copy(ones1r[:], ones1f[:])
            onescf = persist.tile([128, 1], F32, tag="onescf")
            nc.vector.memset(onescf[:], 1.0)
            onescr = persist.tile([128, 1], F32R, tag="onescr")
            nc.vector.tensor_copy(onescr[:], onescf[:])

            def load_vec(handle, name):
                t = persist.tile([128, 2], F32, tag="vec_" + name, name=name + "_sb")
                nc.sync.dma_start(out=t[:], in_=handle.ap().rearrange("(g p) -> p g", g=2))
                return t

            bcbl_sb = load_vec(b_cbl, "bcbl")
            bng_sb = load_vec(bn_gamma, "bng")
            bnb_sb = load_vec(bn_beta, "bnb")
            bq_sb = load_vec(bq, "bq")
            bk_sb = load_vec(bk, "bk")
            bv_sb = load_vec(bv, "bv")
            bqs_sb = persist.tile([128, 2], F32, tag="bqs")
            nc.vector.tensor_scalar_mul(bqs_sb[:], bq_sb[:], 1.0 / 16.0)
            eps_sb = persist.tile([128, 1], F32, tag="eps_sb")
            nc.vector.memset(eps_sb[:], BN_EPS)

            # q/k/v weight mats, c-major halves, f32r
            wqkv_r = {}
            cT = persist.tile([128, 2, NPIX], F32R, tag="cT")
            lng = persist.tile([128, 2, 1024], F32, tag="lng")
            lnb = persist.tile([128, 2, 1024], F32, tag="lnb")

            # =========== conv phase (pools close before attention) ===========
            with tc.tile_pool(name="convp", bufs=1) as convp, \
                 tc.tile_pool(name="cvps", bufs=4, space="PSUM") as cvps:
                xpad = convp.tile([128, 2, S, HP, HP], F32R, tag="xpad")
                wc_r = convp.tile([128, 2, 9, C], F32R, tag="wc_r")
                convraw = convp.tile([128, 2, NPIX], F32, tag="convraw")
                cstat = pstat.tile([128, 2, 8, 6], F32, tag="cstat")

                with tc.tile_pool(name="setup", bufs=1) as setup, \
                     tc.tile_pool(name="tpps", bufs=4, space="PSUM") as tpps:
                    # ---- input transpose: pixel-major -> c-major padded ----
                    xpix = setup.tile([128, 32, C], F32, tag="xpix")
                    nc.sync.dma_start(
                        out=xpix[:], in_=x_s.ap().rearrange("(t p) c -> p t c", p=128))
                    zb = setup.tile([128, S, HP], F32, tag="zb")
                    nc.vector.memset(zb[:], 0.0)
                    for ch in range(2):
                        nc.vector.tensor_copy(xpad[:, ch, :, 0, :], zb[:])
                        nc.vector.tensor_copy(xpad[:, ch, :, HP - 1, :], zb[:])
                        nc.scalar.copy(xpad[:, ch, :, :, 0], zb[:])
                        nc.scalar.copy(xpad[:, ch, :, :, HP - 1], zb[:])
                    for t in range(32):
                        s, tb = divmod(t, 8)
                        r0 = tb * 4
                        for ch in range(2):
                            tp = tpps.tile([128, 128], F32, tag="tp")
                            nc.tensor.transpose(
                                tp[:], xpix[:, t, ch * 128:(ch + 1) * 128], ident[:])
                            ecopy(t * 2 + ch,
                                  xpad[:, ch, s, 1 + r0:5 + r0, 1:1 + W],
                                  tp[:].rearrange("p (a b) -> p a b", b=W))

                    # ---- ln_gamma / ln_beta transpose to c-major ----
                    lns = setup.tile([128, 8, C], F32, tag="lns")
                    for src, dst in ((ln_gamma, lng), (ln_beta, lnb)):
                        nc.sync.dma_start(
                            out=lns[:], in_=src.ap().rearrange("(t p) c -> p t c", p=128))
                        for t in range(8):
                            for ch in range(2):
                                tp = tpps.tile([128, 128], F32, tag="tp")
                                nc.tensor.transpose(
                                    tp[:], lns[:, t, ch * 128:(ch + 1) * 128], ident[:])
                                ecopy(t * 2 + ch, dst[:, ch, t * 128:(t + 1) * 128], tp[:])

                    # ---- conv weights to SBUF (f32r) ----
                    wst = setup.tile([128, 9, C], F32, tag="wst")
                    wdram = w_cbl.ap().rearrange("a b (g p) d -> p (a b) g d", g=2)
                    for ch in range(2):
                        nc.sync.dma_start(out=wst[:], in_=wdram[:, :, ch, :])
                        nc.vector.tensor_copy(wc_r[:, ch, :, :], wst[:])

                    # ---- qkv weights ----
                    wqs = setup.tile([128, 2, C], F32, tag="wqs")
                    for handle, name in ((wq, "wq"), (wk, "wk"), (wv, "wv")):
                        wr = persist.tile([128, 2, C], F32R, tag="wr_" + name,
                                          name=name + "_r")
                        nc.sync.dma_start(
                            out=wqs[:], in_=handle.ap().rearrange("(g p) d -> p g d", g=2))
                        nc.scalar.copy(wr[:], wqs[:])
                        wqkv_r[name] = wr

                # ---- conv matmuls ----
                for dh in range(2):
                    for cp in range(4):
                        pss = [cvps.tile([128, 512], F32, tag="cv", name=f"cv_{dh}_{cp}_{h}")
                               for h in range(2)]
                        for it in range(9):
                            ky, kx = divmod(it, 3)
                            for ch in range(2):
                                lhsT = wc_r[:, ch, it, dh * 128:(dh + 1) * 128]
                                first = (it == 0 and ch == 0)
                                last = (it == 8 and ch == 1)
                                for hf in range(2):
                                    chunk = cp * 2 + hf
                                    s, rbh = divmod(chunk, 2)
                                    rb = rbh * 16
                                    rhs = xpad[:, ch, s, rb + ky:rb + ky + 16, kx:kx + W]
                                    nc.tensor.matmul(pss[hf][:], lhsT, rhs,
                                                     start=first, stop=last)
                        for hf in range(2):
                            chunk = cp * 2 + hf
                            sl = slice(chunk * 512, (chunk + 1) * 512)
                            nc.scalar.activation(
                                convraw[:, dh, sl], pss[hf][:], AF.Identity,
                                bias=bcbl_sb[:, dh:dh + 1], scale=1.0)
                            nc.vector.bn_stats(out=cstat[:, dh, chunk, :],
                                               in_=convraw[:, dh, sl])

                # ---- BN stats: aggregate, AllReduce, scale/shift ----
                mvc = pstat.tile([128, 2, 2], F32, tag="mvc")
                for ch in range(2):
                    nc.vector.bn_aggr(out=mvc[:, ch, :], in_=cstat[:, ch, :, :])
                s12 = pstat.tile([128, 4], F32, tag="s12")
                msq = pstat.tile([128, 2], F32, tag="msq")
                # S1 = mean*npix ; S2 = (var + mean^2)*npix
                nc.vector.tensor_scalar_mul(
                    s12[:, 0:2], mvc[:, :, 0], float(NPIX))
                nc.scalar.activation(msq[:], mvc[:, :, 0], AF.Square)
                nc.vector.tensor_add(msq[:], msq[:], mvc[:, :, 1])
                nc.vector.tensor_scalar_mul(s12[:, 2:4], msq[:], float(NPIX))

                cc_in = dram.tile([128, 4], F32, tag="cc_in")
                cc_out = dram.tile([128, 4], F32, tag="cc_out")
                nc.sync.dma_start(out=cc_in[:], in_=s12[:])
                nc.gpsimd.collective_compute(
                    "AllReduce", OP.add,
                    replica_groups=[list(range(N_CORES))],
                    ins=[cc_in.opt()], outs=[cc_out.opt()])
                g12 = pstat.tile([128, 4], F32, tag="g12")
                nc.sync.dma_start(out=g12[:], in_=cc_out[:])

                NTOT = float(N_CORES * NPIX)
                gmu = pstat.tile([128, 2], F32, tag="gmu")
                nc.vector.tensor_scalar_mul(gmu[:], g12[:, 0:2], 1.0 / NTOT)
                gvar = pstat.tile([128, 2], F32, tag="gvar")
                nc.vector.tensor_scalar_mul(gvar[:], g12[:, 2:4], 1.0 / NTOT)
                gsq = pstat.tile([128, 2], F32, tag="gsq")
                nc.scalar.activation(gsq[:], gmu[:], AF.Square)
                nc.vector.tensor_sub(gvar[:], gvar[:], gsq[:])
                gsd = pstat.tile([128, 2], F32, tag="gsd")
                nc.scalar.activation(gsd[:], gvar[:], AF.Sqrt, bias=eps_sb[:])
                gistd = pstat.tile([128, 2], F32, tag="gistd")
                nc.vector.reciprocal(gistd[:], gsd[:])
                scale_bn = pstat.tile([128, 2], F32, tag="scale_bn")
                nc.vector.tensor_mul(scale_bn[:], bng_sb[:], gistd[:])
                shift_bn = pstat.tile([128, 2], F32, tag="shift_bn")
                nc.vector.tensor_mul(shift_bn[:], gmu[:], scale_bn[:])
                nc.vector.tensor_sub(shift_bn[:], bnb_sb[:], shift_bn[:])

                # ---- BN apply + leaky relu -> cT (f32r) ----
                for ch in range(2):
                    nc.scalar.activation(
                        cT[:, ch, :], convraw[:, ch, :], AF.Prelu,
                        bias=shift_bn[:, ch:ch + 1], scale=scale_bn[:, ch:ch + 1],
                        alpha=ALPHA)

            # =========== attention phase ===========
            with tc.tile_pool(name="attp", bufs=1) as attp, \
                 tc.tile_pool(name="ypool", bufs=1) as ypool:
                qbf = attp.tile([128, 2, NPIX], BF16, tag="qbf")
                kbf = attp.tile([128, 2, NPIX], BF16, tag="kbf")
                v2bf = attp.tile([128, 32, C], BF16, tag="v2bf")

                # ---- q, k, v matmuls ----
                with tc.tile_pool(name="qkps", bufs=4, space="PSUM") as qkps:
                    for dh in range(2):
                        for chunk in range(8):
                            sl = slice(chunk * 512, (chunk + 1) * 512)
                            psq = qkps.tile([128, 512], F32, tag="qk")
                            psk = qkps.tile([128, 512], F32, tag="qk")
                            for ch in range(2):
                                nc.tensor.matmul(
                                    psq[:], wqkv_r["wq"][:, ch, dh * 128:(dh + 1) * 128],
                                    cT[:, ch, sl], start=(ch == 0), stop=(ch == 1))
                                nc.tensor.matmul(
                                    psk[:], wqkv_r["wk"][:, ch, dh * 128:(dh + 1) * 128],
                                    cT[:, ch, sl], start=(ch == 0), stop=(ch == 1))
                            nc.scalar.activation(
                                qbf[:, dh, sl], psq[:], AF.Identity,
                                bias=bqs_sb[:, dh:dh + 1], scale=1.0 / 16.0)
                            nc.vector.tensor_scalar_add(
                                kbf[:, dh, sl], psk[:], bk_sb[:, dh:dh + 1])
                    for jt32 in range(32):
                        psv = qkps.tile([128, 512], F32, tag="qk")
                        for ch in range(2):
                            nc.tensor.matmul(
                                psv[:, 0:C], cT[:, ch, jt32 * 128:(jt32 + 1) * 128],
                                wqkv_r["wv"][:, ch, :], start=(ch == 0), stop=(ch == 1))
                        ecopy(jt32, v2bf[:, jt32, :], psv[:, 0:C])

                # ---- per-sample attention + residual + LN stats ----
                ys = []
                lstat = pstat.tile([128, 8, 2, 6], F32, tag="lstat")
                with tc.tile_pool(name="attps", bufs=3, space="PSUM") as attps, \
                     tc.tile_pool(name="tpo", bufs=2, space="PSUM") as tpo:
                    for s in range(S):
                        Es = []
                        for jt in range(8):
                            sps = attps.tile([128, 1024], F32, tag="big",
                                             name=f"sc_{s}_{jt}")
                            for nh in range(2):
                                for ch in range(2):
                                    nc.tensor.matmul(
                                        sps[:, nh * 512:(nh + 1) * 512],
                                        kbf[:, ch, s * 1024 + jt * 128:s * 1024 + (jt + 1) * 128],
                                        qbf[:, ch, s * 1024 + nh * 512:s * 1024 + (nh + 1) * 512],
                                        start=(ch == 0), stop=(ch == 1))
                            E = attp.tile([128, 1024], BF16, tag="E", bufs=10,
                                          name=f"E_{s}_{jt}")
                            nc.scalar.activation(E[:], sps[:], AF.Exp)
                            Es.append(E)
                        # Z broadcast over partitions: ones^T @ E
                        zps = attps.tile([128, 1024], F32, tag="big", name=f"z_{s}")
                        for jt in range(8):
                            for nh in range(2):
                                nc.tensor.matmul(
                                    zps[:, nh * 512:(nh + 1) * 512], ones_bf[:],
                                    Es[jt][:, nh * 512:(nh + 1) * 512],
                                    start=(jt == 0), stop=(jt == 7))
                        zr = ypool.tile([128, 1024], F32, tag="zr", bufs=1,
                                        name=f"zr_{s}")
                        nc.vector.reciprocal(zr[:], zps[:])
                        for ch in range(2):
                            aps = attps.tile([128, 1024], F32, tag="big",
                                             name=f"at_{s}_{ch}")
                            for jt in range(8):
                                for nh in range(2):
                                    nc.tensor.matmul(
                                        aps[:, nh * 512:(nh + 1) * 512],
                                        v2bf[:, s * 8 + jt, ch * 128:(ch + 1) * 128],
                                        Es[jt][:, nh * 512:(nh + 1) * 512],
                                        start=(jt == 0), stop=(jt == 7))
                            attn = ypool.tile([128, 1024], F32, tag="tmp", bufs=4,
                                              name=f"attn_{s}_{ch}")
                            nc.vector.tensor_mul(attn[:], aps[:], zr[:])
                            y = ypool.tile([128, 1024], F32, tag="y", bufs=8,
                                           name=f"y_{s}_{ch}")
                            nc.gpsimd.tensor_add(
                                y[:], attn[:], cT[:, ch, s * 1024:(s + 1) * 1024].bitcast(F32))
                            for b2 in range(2):
                                nc.vector.bn_stats(
                                    out=lstat[:, s * 2 + ch, b2, :],
                                    in_=y[:, b2 * 512:(b2 + 1) * 512])
                            ys.append(y)

                    # ---- LN scalars (per sample) ----
                    lmv = pstat.tile([128, 8, 2], F32, tag="lmv")
                    for k in range(8):
                        nc.vector.bn_aggr(out=lmv[:, k, :], in_=lstat[:, k, :, :])
                    SC = pstat.tile([128, 16], F32R, tag="SC")
                    lms = pstat.tile([128, 8], F32, tag="lms")
                    nc.scalar.activation(lms[:], lmv[:, :, 0], AF.Square)
                    nc.vector.tensor_add(lms[:], lms[:], lmv[:, :, 1])
                    nc.vector.tensor_scalar_mul(SC[:, 0:8], lmv[:, :, 0], 1024.0)
                    nc.vector.tensor_scalar_mul(SC[:, 8:16], lms[:], 1024.0)
                    Tb = attps.tile([128, 1024], F32, tag="big", name="Tb")
                    nc.tensor.matmul(Tb[0:1, 0:16], onescr[:], SC[:],
                                     start=True, stop=True)
                    Tr = pstat.tile([1, 8], F32, tag="Tr")
                    nc.vector.reduce_sum(
                        Tr[:], Tb[0:1, 0:16].rearrange("p (a b) -> p a b", b=2),
                        axis=mybir.AxisListType.X)
                    NLN = float(H * W * C)
                    mu4 = pstat.tile([1, 4], F32, tag="mu4")
                    nc.vector.tensor_scalar_mul(mu4[:], Tr[:, 0:4], 1.0 / NLN)
                    ex4 = pstat.tile([1, 4], F32, tag="ex4")
                    nc.vector.tensor_scalar_mul(ex4[:], Tr[:, 4:8], 1.0 / NLN)
                    ms4 = pstat.tile([1, 4], F32, tag="ms4")
                    nc.scalar.activation(ms4[:], mu4[:], AF.Square)
                    nc.vector.tensor_sub(ex4[:], ex4[:], ms4[:])
                    sd4 = pstat.tile([1, 4], F32, tag="sd4")
                    nc.scalar.activation(sd4[:], ex4[:], AF.Sqrt, bias=eps_sb[0:1, :])
                    is4 = pstat.tile([1, 4], F32, tag="is4")
                    nc.vector.reciprocal(is4[:], sd4[:])
                    bcs = pstat.tile([1, 8], F32R, tag="bcs")
                    nc.vector.tensor_copy(bcs[:, 0:4], is4[:])
                    nc.vector.tensor_copy(bcs[:, 4:8], mu4[:])
                    B8 = attps.tile([128, 1024], F32, tag="big", name="B8")
                    nc.tensor.matmul(B8[:, 0:8], ones1r[:], bcs[:],
                                     start=True, stop=True)
                    musd = pstat.tile([128, 8], F32, tag="musd")
                    nc.vector.tensor_copy(musd[:], B8[:, 0:8])
                    # istd8 / s2t with columns (2s+ch)
                    istd8 = pstat.tile([128, 8], F32, tag="istd8")
                    s2t = pstat.tile([128, 8], F32, tag="s2t")
                    mux8 = pstat.tile([128, 8], F32, tag="mux8")
                    for ch in range(2):
                        nc.vector.tensor_copy(istd8[:, ch:ch + 7:2], musd[:, 0:4])
                        nc.vector.tensor_copy(mux8[:, ch:ch + 7:2], musd[:, 4:8])
                        for s in range(S):
                            nc.vector.tensor_copy(
                                s2t[:, s * 2 + ch:s * 2 + ch + 1], bv_sb[:, ch:ch + 1])
                    # s2t currently: bv columns broadcast; fix: s2t = (bv - mu)*istd
                    nc.vector.tensor_sub(s2t[:], s2t[:], mux8[:])
                    nc.vector.tensor_mul(s2t[:], s2t[:], istd8[:])

                    # ---- final normalize + gamma/beta + leaky + transpose out ----
                    outst = attp.tile([128, 8, C], F32, tag="outst", bufs=2)
                    for s in range(S):
                        for ch in range(2):
                            k = s * 2 + ch
                            y = ys[k]
                            yn = ypool.tile([128, 1024], F32, tag="tmp", bufs=4,
                                            name=f"yn_{k}")
                            nc.vector.tensor_scalar(
                                out=yn[:], in0=y[:],
                                scalar1=istd8[:, k:k + 1], scalar2=s2t[:, k:k + 1],
                                op0=OP.mult, op1=OP.add)
                            yg = ypool.tile([128, 1024], F32, tag="tmp", bufs=4,
                                            name=f"yg_{k}")
                            nc.gpsimd.tensor_mul(yg[:], yn[:], lng[:, ch, :])
                            nc.gpsimd.tensor_add(yg[:], yg[:], lnb[:, ch, :])
                            yo = ypool.tile([128, 1024], F32, tag="yo", bufs=2,
                                            name=f"yo_{k}")
                            nc.scalar.activation(yo[:], yg[:], AF.Prelu, alpha=ALPHA)
                            for t in range(8):
                                tp = tpo.tile([128, 128], F32, tag="tpo")
                                nc.tensor.transpose(
                                    tp[:], yo[:, t * 128:(t + 1) * 128], ident[:])
                                ecopy(t, outst[:, t, ch * 128:(ch + 1) * 128], tp[:])
                        nc.sync.dma_start(
                            out=y_s.ap()[s * 1024:(s + 1) * 1024, :].rearrange(
                                "(t p) c -> p t c", p=128),
                            in_=outst[:])

    nc.compile()
    return nc


def _get_nc():
    if "nc" not in _CACHE:
        _CACHE["nc"] = _build()
    return _CACHE["nc"]


def kernel(**inputs):
    from concourse.bass_utils import run_bass_kernel_spmd

    nc = _get_nc()
    x = np.ascontiguousarray(inputs["x"], dtype=np.float32)          # (32,32,32,256)
    shared = {
        "w_cbl": np.ascontiguousarray(inputs["w_cbl"], np.float32),
        "b_cbl": np.ascontiguousarray(inputs["b_cbl"], np.float32),
        "bn_gamma": np.ascontiguousarray(inputs["bn_gamma"], np.float32),
        "bn_beta": np.ascontiguousarray(inputs["bn_beta"], np.float32),
        "wq": np.ascontiguousarray(inputs["wq"], np.float32),
        "bq": np.ascontiguousarray(inputs["bq"], np.float32),
        "wk": np.ascontiguousarray(inputs["wk"], np.float32),
        "bk": np.ascontiguousarray(inputs["bk"], np.float32),
        "wv": np.ascontiguousarray(inputs["wv"], np.float32),
        "bv": np.ascontiguousarray(inputs["bv"], np.float32),
        "ln_gamma": np.ascontiguousarray(inputs["ln_gamma"], np.float32).reshape(H * W, C),
        "ln_beta": np.ascontiguousarray(inputs["ln_beta"], np.float32).reshape(H * W, C),
    }
    in_maps = []
    for i in range(N_CORES):
        m = dict(shared)
        m["x_s"] = x[i * S:(i + 1) * S].reshape(NPIX, C)
        in_maps.append(m)

    res = run_bass_kernel_spmd(nc, in_maps, list(range(N_CORES)))
    _CACHE["last_results"] = res
    out = np.empty((N_CORES * S, H, W, C), np.float32)
    for i in range(N_CORES):
        out[i * S:(i + 1) * S] = res.results[i]["y_s"].reshape(S, H, W, C)
    return out


def kernel_traced(**inputs):
    """Like kernel() but with NTFF tracing; returns (out, exec_time_ns)."""
    from concourse.bass_utils import run_bass_kernel_spmd

    nc = _get_nc()
    x = np.ascontiguousarray(inputs["x"], dtype=np.float32)
    shared = {k: np.ascontiguousarray(inputs[k], np.float32)
              for k in ("w_cbl", "b_cbl", "bn_gamma", "bn_beta", "wq", "bq",
                        "wk", "bk", "wv", "bv")}
    shared["ln_gamma"] = np.ascontiguousarray(inputs["ln_gamma"], np.float32).reshape(H * W, C)
    shared["ln_beta"] = np.ascontiguousarray(inputs["ln_beta"], np.float32).reshape(H * W, C)
    in_maps = []
    for i in range(N_CORES):
        m = dict(shared)
        m["x_s"] = x[i * S:(i + 1) * S].reshape(NPIX, C)
        in_maps.append(m)
    res = run_bass_kernel_spmd(nc, in_maps, list(range(N_CORES)), trace=True)
    _CACHE["last_results"] = res
    out = np.empty((N_CORES * S, H, W, C), np.float32)
    for i in range(N_CORES):
        out[i * S:(i + 1) * S] = res.results[i]["y_s"].reshape(S, H, W, C)
    return out, res.exec_time_ns
